# revision 1
# baseline (speedup 1.0000x reference)
"""DAGNN (10-hop propagation + sigmoid gating) Bass kernel for 8 trn2 NeuronCores.

Strategy (1D node partition, SPMD-uniform schedule):
  - Host assigns nodes to (core, window, slot) with degree balancing so every
    core runs an identical instruction stream (one NEFF, 8 cores).
  - Node features live in a Shared DRAM replica as [hi|lo] bf16 pairs (256B
    rows); hi+lo reconstructs fp32 exactly to ~2^-18 relative.
  - Per hop: dma_gather pulls per-edge rows; PE computes the segment-sum via
    one-hot indicator matmuls (indicator built on DVE with tensor_scalar
    is_equal against precomputed per-edge keys) accumulating in fp32 PSUM;
    DVE drains PSUM, applies deg^-1 scaling and re-splits hi/lo; AllGather
    rebuilds the replica for the next hop.
  - Final phase: sigmoid gates from the 11 archived per-hop slices.
"""

import sys

sys.path.insert(0, "/opt/trn_rl_repo")

import numpy as np
import ml_dtypes

# ----------------------------------------------------------------------------
# Problem constants (hardcoded per spec nn_DAGNNConv_1846835938000).
# _config() recomputes the derived values; the sim test uses a small config.
# ----------------------------------------------------------------------------
BF16 = ml_dtypes.bfloat16


def _config(n_nodes, k_hops, n_cores, w_per_core, w_per_super, t_per_bucket):
    g = globals()
    g["N_NODES"] = n_nodes
    g["D"] = 64
    g["K_HOPS"] = k_hops
    g["N_CORES"] = n_cores
    g["WIN"] = 128
    g["W_PER_CORE"] = w_per_core
    assert w_per_core * n_cores * 128 >= n_nodes
    g["ROWS_PC"] = w_per_core * 128
    g["REP_ROWS"] = n_cores * g["ROWS_PC"]
    g["N_SRC_WIN"] = 4
    assert g["REP_ROWS"] % 4 == 0
    g["SRC_WIN"] = g["REP_ROWS"] // 4
    assert g["SRC_WIN"] <= 32768
    g["W_PER_SUPER"] = w_per_super
    assert w_per_core % w_per_super == 0
    assert w_per_super <= 8
    g["SUPERS"] = w_per_core // w_per_super
    g["T_PER_BUCKET"] = t_per_bucket
    g["SLOTS_PER_WS"] = t_per_bucket * 128
    g["BUCKET_SLOTS"] = w_per_super * g["SLOTS_PER_WS"]
    g["SLOTS_TOTAL"] = w_per_core * 4 * g["SLOTS_PER_WS"]
    g["TILES_TOTAL"] = g["SLOTS_TOTAL"] // 128


_config(100000, 10, 8, 104, 4, 3)


# ----------------------------------------------------------------------------
# Host preprocessing
# ----------------------------------------------------------------------------
def _balance_assign(deg_s_fn, tot):
    """Assign nodes to global windows (N_CORES*W_PER_CORE, cap 128 each) so
    that every (window, src-window) edge count stays <= SLOTS_PER_WS.

    deg_s_fn(core_of, wloc_of, pos_of) -> [n, N_SRC_WIN] per-node edge counts
    by src window under the *current* assignment (src windows depend on the
    permutation, so we iterate LPT + repair rounds).
    """
    import heapq

    n = tot.shape[0]
    n_windows = N_CORES * W_PER_CORE
    order = np.argsort(-tot, kind="stable")
    heap = [(0, w) for w in range(n_windows)]
    heapq.heapify(heap)
    win_of = np.empty(n, dtype=np.int32)
    win_fill = np.zeros(n_windows, dtype=np.int32)
    for v in order:
        while True:
            load, w = heapq.heappop(heap)
            if win_fill[w] < WIN:
                break
        win_of[v] = w
        win_fill[w] += 1
        if win_fill[w] < WIN:
            heapq.heappush(heap, (load + int(tot[v]), w))

    rng = np.random.default_rng(12345)
    cap = SLOTS_PER_WS
    for round_i in range(12):
        # positions within windows: stable order by node id
        pos_of = np.zeros(n, dtype=np.int32)
        ordv = np.lexsort((np.arange(n), win_of))
        posctr = np.zeros(n_windows, dtype=np.int32)
        for v in ordv:
            pos_of[v] = posctr[win_of[v]]
            posctr[win_of[v]] += 1
        core_of = (win_of // W_PER_CORE).astype(np.int32)
        wloc_of = (win_of % W_PER_CORE).astype(np.int32)
        deg_s = deg_s_fn(core_of, wloc_of, pos_of)  # [n, 4]
        loads = np.zeros((n_windows, N_SRC_WIN), dtype=np.int64)
        np.add.at(loads, win_of, deg_s)
        over = np.flatnonzero((loads > cap).any(axis=1))
        if len(over) == 0:
            return core_of, wloc_of, pos_of
        # repair: move high-deg nodes out of overloaded windows
        maxload = loads.max(axis=1)
        for w in over:
            s_bad = int(np.argmax(loads[w]))
            excess = int(loads[w, s_bad] - cap)
            members = np.flatnonzero(win_of == w)
            mdeg = deg_s[members, s_bad]
            for v in members[np.argsort(-mdeg)]:
                if excess <= 0:
                    break
                # candidate windows with node space and slack
                cands = rng.integers(0, n_windows, 64)
                best, bestval = -1, None
                for cw in cands:
                    if cw == w or posctr[cw] >= WIN:
                        continue
                    val = int((loads[cw] + deg_s[v]).max())
                    if val <= cap - 8 and (bestval is None or val < bestval):
                        best, bestval = int(cw), val
                if best < 0:
                    continue
                loads[w] -= deg_s[v]
                loads[best] += deg_s[v]
                win_of[v] = best
                posctr[w] -= 1
                posctr[best] += 1
                excess -= int(deg_s[v, s_bad])
    raise RuntimeError("balance repair failed to converge")


def _preprocess(feats, s, src, dst):
    src = np.asarray(src, dtype=np.int64)
    dst = np.asarray(dst, dtype=np.int64)
    n = N_NODES
    deg = np.bincount(dst, minlength=n).astype(np.float64)
    norm = (deg ** -0.5).astype(np.float32)
    n2 = (1.0 / deg).astype(np.float32)
    sqrtdeg = np.sqrt(deg).astype(np.float32)

    # ---- peel one self-loop per node (handled via identity matmul) ----
    loop_mask = src == dst
    loop_idx = np.flatnonzero(loop_mask)
    uniq_nodes, first_pos = np.unique(dst[loop_idx], return_index=True)
    if len(uniq_nodes) != n:
        raise RuntimeError("not every node has a self-loop; identity fold invalid")
    drop = np.zeros(len(src), dtype=bool)
    drop[loop_idx[first_pos]] = True
    src = src[~drop]
    dst = dst[~drop]

    # ---- node assignment (core, window, pos) ----
    deg_r = np.bincount(dst, minlength=n).astype(np.int64)  # random-edge degree

    def deg_s_fn(core_of, wloc_of, pos_of):
        rep_row = (
            core_of.astype(np.int64) * ROWS_PC
            + pos_of.astype(np.int64) * W_PER_CORE
            + wloc_of.astype(np.int64)
        )
        es = rep_row[src] // SRC_WIN
        out = np.zeros((n, N_SRC_WIN), dtype=np.int64)
        np.add.at(out, (dst, es), 1)
        return out

    core_of, wloc_of, pos_of = _balance_assign(deg_s_fn, deg_r)
    # replica row of node v: c*ROWS_PC + p*W_PER_CORE + w  (partition-major)
    rep_row = core_of.astype(np.int64) * ROWS_PC + pos_of.astype(np.int64) * W_PER_CORE + wloc_of.astype(np.int64)

    # ---- per-core edge bucketing ----
    e_core = core_of[dst]
    e_w = wloc_of[dst]            # window of dst within core
    e_key = pos_of[dst]           # indicator column = position of dst in window
    e_srow = rep_row[src]         # replica row of src
    e_s = e_srow // SRC_WIN       # src window id (0..3)
    e_gidx = (e_srow - e_s * SRC_WIN).astype(np.int64)  # int16-safe

    gidx_all = np.zeros((N_CORES, SLOTS_TOTAL), dtype=np.int16)
    keys_all = np.full((N_CORES, SLOTS_TOTAL), -1.0, dtype=BF16)

    overflow = 0
    for c in range(N_CORES):
        m = e_core == c
        cw = e_w[m]
        cs = e_s[m]
        ckey = e_key[m]
        cg = e_gidx[m]
        # slot base for (w, s): bucket (super=w//14, s) at
        # base = ((w//14)*4 + s)*BUCKET_SLOTS + (w%14)*SLOTS_PER_WS
        ws = cw * N_SRC_WIN + cs
        order = np.argsort(ws, kind="stable")
        cw, cs, ckey, cg, ws = cw[order], cs[order], ckey[order], cg[order], ws[order]
        counts = np.bincount(ws, minlength=W_PER_CORE * N_SRC_WIN)
        if counts.max() > SLOTS_PER_WS:
            overflow = max(overflow, int(counts.max()))
            continue
        starts = np.zeros_like(counts)
        # slot start of each (w, s) run
        w_arr = np.arange(W_PER_CORE * N_SRC_WIN) // N_SRC_WIN
        s_arr = np.arange(W_PER_CORE * N_SRC_WIN) % N_SRC_WIN
        starts = (
            ((w_arr // W_PER_SUPER) * N_SRC_WIN + s_arr) * BUCKET_SLOTS
            + (w_arr % W_PER_SUPER) * SLOTS_PER_WS
        )
        # position within run
        runpos = np.arange(len(ws)) - np.repeat(
            np.concatenate([[0], np.cumsum(counts)[:-1]]), counts
        )
        slots = starts[ws] + runpos
        gidx_all[c, slots] = cg.astype(np.int16)
        keys_all[c, slots] = ckey.astype(BF16)

    if overflow:
        raise RuntimeError(f"bucket overflow: {overflow} > {SLOTS_PER_WS}")

    # ---- initial replica g0 = norm * feats, hi/lo packed, partition-major ----
    g0 = feats.astype(np.float32) * norm[:, None]
    hi = g0.astype(BF16)
    lo = (g0 - hi.astype(np.float32)).astype(BF16)
    packed = np.concatenate([hi, lo], axis=1)  # [N, 128] bf16
    g0_rep = np.zeros((REP_ROWS, 2 * D), dtype=BF16)
    g0_rep[rep_row] = packed

    # per-core tables in (p, w) layout
    def pw_table(vec):  # vec [N] -> [N_CORES, 128, W_PER_CORE]
        out = np.zeros((N_CORES, WIN, W_PER_CORE), dtype=np.float32)
        out[core_of, pos_of, wloc_of] = vec
        return out

    n2_pw = pw_table(n2)
    n2_pw[n2_pw == 0] = 1.0
    sqd_pw = pw_table(sqrtdeg)

    # wrapped gather index layout: idx j -> (j%16, j//16), replicated to all
    # 8 gpsimd 16-partition groups -> [128, SLOTS_TOTAL//16]
    gidx_wrapped = np.ascontiguousarray(
        np.tile(
            gidx_all.reshape(N_CORES, SLOTS_TOTAL // 16, 16).transpose(0, 2, 1),
            (1, 8, 1),
        )
    )
    # keys layout [128, TILES_TOTAL]: slot j -> (j%128, j//128)
    keys_tiles = np.ascontiguousarray(
        keys_all.reshape(N_CORES, TILES_TOTAL, 128).transpose(0, 2, 1)
    )

    iota = np.broadcast_to(np.arange(128, dtype=np.float32), (128, 128)).astype(BF16)
    iota = np.ascontiguousarray(iota)
    s_bcast = np.broadcast_to(np.asarray(s, dtype=np.float32).reshape(1, D), (128, D))
    s_bcast = np.ascontiguousarray(s_bcast)

    in_maps = []
    for c in range(N_CORES):
        in_maps.append(
            {
                "g0_own": np.ascontiguousarray(
                    g0_rep[c * ROWS_PC : (c + 1) * ROWS_PC]
                ),
                "gidx": gidx_wrapped[c],
                "keys": keys_tiles[c],
                "n2_pw": np.ascontiguousarray(n2_pw[c]),
                "sqd_pw": np.ascontiguousarray(sqd_pw[c]),
                "s_bcast": s_bcast,
                "iota": iota,
            }
        )
    meta = {
        "core_of": core_of,
        "wloc_of": wloc_of,
        "pos_of": pos_of,
    }
    return in_maps, meta


# ----------------------------------------------------------------------------
# Bass kernel builder (identical program for all cores)
# ----------------------------------------------------------------------------
def _build():
    import concourse.bacc as bacc
    import concourse.mybir as mybir
    from concourse.tile import TileContext

    fp32 = mybir.dt.float32
    bf16 = mybir.dt.bfloat16
    i16 = mybir.dt.int16

    nc = bacc.Bacc(None, target_bir_lowering=False, num_devices=N_CORES, num_swdge_queues=4)

    # I/O
    g0_own = nc.dram_tensor("g0_own", [ROWS_PC, 2 * D], bf16, kind="ExternalInput")
    gidx_in = nc.dram_tensor("gidx", [128, SLOTS_TOTAL // 16], i16, kind="ExternalInput")
    keys_in = nc.dram_tensor("keys", [128, TILES_TOTAL], bf16, kind="ExternalInput")
    n2_in = nc.dram_tensor("n2_pw", [128, W_PER_CORE], fp32, kind="ExternalInput")
    sqd_in = nc.dram_tensor("sqd_pw", [128, W_PER_CORE], fp32, kind="ExternalInput")
    s_in = nc.dram_tensor("s_bcast", [128, D], fp32, kind="ExternalInput")
    iota_in = nc.dram_tensor("iota", [128, 128], bf16, kind="ExternalInput")
    out_pm = nc.dram_tensor("out_pm", [ROWS_PC, D], fp32, kind="ExternalOutput")

    # hop buffers: cc_in[k] per-core slice (k=0..K), cc_out[k] shared replica
    cc_in = [
        nc.dram_tensor(f"cc_in_{k}", [ROWS_PC, 2 * D], bf16) for k in range(K_HOPS + 1)
    ]
    cc_out = [
        nc.dram_tensor(f"cc_out_{k}", [REP_ROWS, 2 * D], bf16, addr_space="Shared")
        for k in range(K_HOPS)
    ]
    groups = [list(range(N_CORES))]

    with TileContext(nc) as tc:
        with tc.tile_pool(name="const", bufs=1) as const_pool:
            # ---- load static tables ----
            gidx_sb = const_pool.tile([128, SLOTS_TOTAL // 16], i16, tag="gidx")
            nc.sync.dma_start(out=gidx_sb[:, :], in_=gidx_in[:, :])
            keys_sb = const_pool.tile([128, TILES_TOTAL], bf16, tag="keys")
            nc.sync.dma_start(out=keys_sb[:, :], in_=keys_in[:, :])
            n2_sb = const_pool.tile([128, W_PER_CORE], fp32, tag="n2")
            nc.sync.dma_start(out=n2_sb[:, :], in_=n2_in[:, :])
            sqd_sb = const_pool.tile([128, W_PER_CORE], fp32, tag="sqd")
            nc.sync.dma_start(out=sqd_sb[:, :], in_=sqd_in[:, :])
            s_sb = const_pool.tile([128, D], fp32, tag="svec")
            nc.sync.dma_start(out=s_sb[:, :], in_=s_in[:, :])
            iota_sb = const_pool.tile([128, 128], bf16, tag="iota")
            nc.sync.dma_start(out=iota_sb[:, :], in_=iota_in[:, :])
            # identity (bf16) for the self-loop fold: built from iota on device
            ident_sb = const_pool.tile([128, 128], bf16, tag="ident")
            pidx_sb = const_pool.tile([128, 1], fp32, tag="pidx")
            nc.gpsimd.iota(
                pidx_sb[:, :],
                [[1, 1]],
                base=0,
                channel_multiplier=1,
                allow_small_or_imprecise_dtypes=True,
            )
            nc.vector.tensor_scalar(
                ident_sb[:, :],
                iota_sb[:, :],
                pidx_sb[:, :],
                None,
                mybir.AluOpType.is_equal,
            )

            with (
                tc.tile_pool(name="chunks", bufs=9) as chunk_pool,
                tc.tile_pool(name="inds", bufs=9) as ind_pool,
                tc.tile_pool(name="stage", bufs=2) as stage_pool,
                tc.tile_pool(name="drain", bufs=4) as drain_pool,
                tc.tile_pool(name="psum", bufs=8, space="PSUM") as psum_pool,
            ):

                # ---- bootstrap: cc_in[0] = g0_own; AllGather -> cc_out[0] ----
                nc.sync.dma_start(out=cc_in[0][:, :], in_=g0_own[:, :])
                nc.gpsimd.collective_compute(
                    "AllGather",
                    mybir.AluOpType.bypass,
                    replica_groups=groups,
                    ins=[cc_in[0][:, :]],
                    outs=[cc_out[0][:, :]],
                )
                # staged_prev holds own g_hop in (p, w, [hi|lo]) layout
                staged_prev = stage_pool.tile(
                    [128, W_PER_CORE, 2 * D], bf16, tag="staged"
                )
                nc.sync.dma_start(
                    out=staged_prev[:, :, :],
                    in_=g0_own[:, :].rearrange("(p w) f -> p w f", p=128),
                )

                # ---- hop loop ----
                def drain_super(sup, banks, staged, cc_dst):
                    for wi in range(W_PER_SUPER):
                        w = sup * W_PER_SUPER + wi
                        bank = banks[wi]
                        his = drain_pool.tile([128, D], fp32, tag="his")
                        nc.scalar.activation(
                            his[:, :],
                            bank[:, 0:D],
                            mybir.ActivationFunctionType.Copy,
                            scale=n2_sb[:, w : w + 1],
                        )
                        los = drain_pool.tile([128, D], fp32, tag="los")
                        nc.scalar.activation(
                            los[:, :],
                            bank[:, D : 2 * D],
                            mybir.ActivationFunctionType.Copy,
                            scale=n2_sb[:, w : w + 1],
                        )
                        g = drain_pool.tile([128, D], fp32, tag="g")
                        nc.vector.tensor_tensor(
                            g[:, :], his[:, :], los[:, :], mybir.AluOpType.add
                        )
                        nc.vector.tensor_copy(out=staged[:, w, 0:D], in_=g[:, :])
                        hif = drain_pool.tile([128, D], fp32, tag="hif")
                        nc.vector.tensor_copy(out=hif[:, :], in_=staged[:, w, 0:D])
                        nc.vector.tensor_tensor(
                            staged[:, w, D : 2 * D],
                            g[:, :],
                            hif[:, :],
                            mybir.AluOpType.subtract,
                        )

                for hop in range(K_HOPS):
                    src_rep = cc_out[hop]
                    staged = stage_pool.tile(
                        [128, W_PER_CORE, 2 * D], bf16, tag="staged"
                    )
                    wt = W_PER_SUPER * T_PER_BUCKET
                    pending = None  # (sup, banks) awaiting drain
                    for sup in range(SUPERS):
                        # gather the 4 src-window chunks of this super
                        chunks = []
                        for s in range(N_SRC_WIN):
                            ch = chunk_pool.tile(
                                [128, BUCKET_SLOTS // 128, 2 * D], bf16, tag="chunk"
                            )
                            bucket = sup * N_SRC_WIN + s
                            col0 = bucket * (BUCKET_SLOTS // 16)
                            row0 = s * SRC_WIN
                            nc.gpsimd.dma_gather(
                                ch[:, :, :],
                                src_rep[row0 : row0 + SRC_WIN, :],
                                gidx_sb[:, col0 : col0 + BUCKET_SLOTS // 16],
                                BUCKET_SLOTS,
                                BUCKET_SLOTS,
                                2 * D,
                                single_packet=False,
                                queue_num=s,
                            )
                            chunks.append(ch)
                        banks = [
                            psum_pool.tile([128, 128], fp32, tag="bank", name="bank")
                            for _ in range(W_PER_SUPER)
                        ]
                        for s in range(N_SRC_WIN):
                            # batched indicator build for the whole bucket:
                            # ind[p, j, f] = (keys[p, col0+j] == f)
                            col0 = (
                                (sup * N_SRC_WIN + s) * W_PER_SUPER
                            ) * T_PER_BUCKET
                            indb = ind_pool.tile([128, wt, 128], bf16, tag="ind")
                            nc.vector.tensor_tensor(
                                indb[:, :, :],
                                iota_sb[:, :]
                                .rearrange("p (one f) -> p one f", one=1)
                                .broadcast_to((128, wt, 128)),
                                keys_sb[:, col0 : col0 + wt].broadcast_to(
                                    (128, wt, 128)
                                ),
                                mybir.AluOpType.is_equal,
                            )
                            for wi in range(W_PER_SUPER):
                                w = sup * W_PER_SUPER + wi
                                bank = banks[wi]
                                for t in range(T_PER_BUCKET):
                                    nc.tensor.matmul(
                                        bank[:, :],
                                        indb[:, wi * T_PER_BUCKET + t, :],
                                        chunks[s][:, wi * T_PER_BUCKET + t, :],
                                        start=(s == 0 and t == 0),
                                        stop=False,
                                    )
                                if s == N_SRC_WIN - 1:
                                    # self-loop fold: psum += I @ g_prev[window]
                                    nc.tensor.matmul(
                                        bank[:, :],
                                        ident_sb[:, :],
                                        staged_prev[:, w, :],
                                        start=False,
                                        stop=True,
                                    )
                        # drain the PREVIOUS super (lag-1 keeps DVE off the
                        # critical path: this super's indicators are already
                        # queued ahead of the drains)
                        if pending is not None:
                            drain_super(pending[0], pending[1], staged, cc_in[hop + 1])
                        pending = (sup, banks)
                    drain_super(pending[0], pending[1], staged, cc_in[hop + 1])
                    nc.sync.dma_start(
                        out=cc_in[hop + 1][:, :].rearrange(
                            "(p w) f -> p (w f)", p=128
                        ),
                        in_=staged[:, :, :],
                    )
                    if hop < K_HOPS - 1:
                        nc.gpsimd.collective_compute(
                            "AllGather",
                            mybir.AluOpType.bypass,
                            replica_groups=groups,
                            ins=[cc_in[hop + 1][:, :]],
                            outs=[cc_out[hop + 1][:, :]],
                        )
                    staged_prev = staged

            # ---- final phase: gating (hop pools closed; reuse the space) ----
            with tc.tile_pool(name="final", bufs=1) as fin_pool:
                zt = fin_pool.tile([128, W_PER_CORE, K_HOPS + 1], fp32, tag="zt")
                acc = fin_pool.tile([128, W_PER_CORE, D], fp32, tag="acc")
                ufull = fin_pool.tile([128, W_PER_CORE, D], fp32, tag="ufull")
                # pass 1: z~[p, w, k] = sum_f u_k[p, w, f] * s[f]
                for k in range(K_HOPS + 1):
                    ck = fin_pool.tile(
                        [128, W_PER_CORE, 2 * D], bf16, tag="ck", bufs=2
                    )
                    nc.sync.dma_start(
                        out=ck[:, :, :],
                        in_=cc_in[k][:, :].rearrange("(p w) f -> p w f", p=128),
                    )
                    nc.vector.tensor_tensor(
                        ufull[:, :, :],
                        ck[:, :, 0:D],
                        ck[:, :, D : 2 * D],
                        mybir.AluOpType.add,
                    )
                    # ufull *= s (s broadcast along windows), in place
                    for w in range(W_PER_CORE):
                        nc.vector.tensor_tensor(
                            ufull[:, w, :],
                            ufull[:, w, :],
                            s_sb[:, :],
                            mybir.AluOpType.mult,
                        )
                    nc.vector.tensor_reduce(
                        zt[:, :, k : k + 1].rearrange("p w one -> p (w one)"),
                        ufull[:, :, :],
                        mybir.AxisListType.X,
                        mybir.AluOpType.add,
                    )
                # sigma = sigmoid(z~ * sqrtdeg) ; scale per (p, w)
                sig = fin_pool.tile([128, W_PER_CORE, K_HOPS + 1], fp32, tag="sig")
                for w in range(W_PER_CORE):
                    nc.vector.tensor_scalar_mul(
                        sig[:, w, :], zt[:, w, :], sqd_sb[:, w : w + 1]
                    )
                nc.scalar.activation(
                    sig[:, :, :],
                    sig[:, :, :],
                    mybir.ActivationFunctionType.Sigmoid,
                )
                # pass 2: acc = sum_k sigma_k * u_k
                nc.vector.memset(acc[:, :, :], 0.0)
                for k in range(K_HOPS + 1):
                    ck = fin_pool.tile(
                        [128, W_PER_CORE, 2 * D], bf16, tag="ck", bufs=2
                    )
                    nc.sync.dma_start(
                        out=ck[:, :, :],
                        in_=cc_in[k][:, :].rearrange("(p w) f -> p w f", p=128),
                    )
                    nc.vector.tensor_tensor(
                        ufull[:, :, :],
                        ck[:, :, 0:D],
                        ck[:, :, D : 2 * D],
                        mybir.AluOpType.add,
                    )
                    for w in range(W_PER_CORE):
                        nc.vector.scalar_tensor_tensor(
                            acc[:, w, :],
                            ufull[:, w, :],
                            sig[:, w, k : k + 1],
                            acc[:, w, :],
                            mybir.AluOpType.mult,
                            mybir.AluOpType.add,
                        )
                # out = acc * sqrtdeg, reusing ufull as the staging tile
                for w in range(W_PER_CORE):
                    nc.vector.tensor_scalar_mul(
                        ufull[:, w, :], acc[:, w, :], sqd_sb[:, w : w + 1]
                    )
                nc.sync.dma_start(
                    out=out_pm[:, :].rearrange("(p w) f -> p (w f)", p=128),
                    in_=ufull[:, :, :],
                )

    nc.finalize()
    return nc


# ----------------------------------------------------------------------------
# Entry point
# ----------------------------------------------------------------------------
_CACHED = {}


def kernel(**inputs):
    feats = np.asarray(inputs["feats"], dtype=np.float32)
    s = np.asarray(inputs["s"], dtype=np.float32)
    src = np.asarray(inputs["src"])
    dst = np.asarray(inputs["dst"])

    in_maps, meta = _preprocess(feats, s, src, dst)

    from concourse.bass_utils import run_bass_kernel_spmd

    nc = _CACHED.get("nc")
    if nc is None:
        nc = _build()
        _CACHED["nc"] = nc

    res = run_bass_kernel_spmd(nc, in_maps, core_ids=list(range(N_CORES)))
    _CACHED["last_result"] = res
    # unshard: out_pm rows p*W_PER_CORE + w correspond to node at
    # (core, window w, pos p)
    out = np.zeros((N_NODES, D), dtype=np.float32)
    core_of, wloc_of, pos_of = meta["core_of"], meta["wloc_of"], meta["pos_of"]
    rows = pos_of.astype(np.int64) * W_PER_CORE + wloc_of.astype(np.int64)
    for c in range(N_CORES):
        m = core_of == c
        out[m] = res.results[c]["out_pm"][rows[m]]
    return out


if __name__ == "__main__":
    # smoke: build only
    nc = _build()
    print("build ok, instructions:", sum(1 for _ in nc.m.functions[0].instructions))



# revision 3
# speedup vs baseline: 1.5548x; 1.5548x over previous
"""DAGNN (10-hop propagation + sigmoid gating) Bass kernel for 8 trn2 NeuronCores.

Strategy (1D node partition, SPMD-uniform schedule):
  - Host assigns nodes to (core, window, slot) with degree balancing so every
    core runs an identical instruction stream (one NEFF, 8 cores).
  - Node features live in a Shared DRAM replica as 256B rows: cols 0:64 hold
    bf16 values, cols 64:128 are never read (gather elem_size must be a
    multiple of 256B).
  - Per hop: dma_gather pulls per-edge rows (2 supers = 8 windows merged per
    gather instruction to amortize gpsimd descriptor-gen overhead); PE
    computes the segment-sum via one-hot indicator matmuls (64-col moving
    operand) accumulating in fp32 PSUM; the Scalar engine drains PSUM with
    deg^-1 scaling straight to bf16; AllGather rebuilds the replica.
  - Gating is fused into the hop loop: z_k = sum_f g_k*s, sig_k =
    sigmoid(z_k*sqrt(deg)), acc += sig_k * g_k — no final reload pass.
"""

import sys

sys.path.insert(0, "/opt/trn_rl_repo")

import numpy as np
import ml_dtypes

BF16 = ml_dtypes.bfloat16


def _config(n_nodes, k_hops, n_cores, w_per_core, w_per_super, t_per_bucket):
    g = globals()
    g["N_NODES"] = n_nodes
    g["D"] = 64
    g["K_HOPS"] = k_hops
    g["N_CORES"] = n_cores
    g["WIN"] = 128
    g["W_PER_CORE"] = w_per_core
    assert w_per_core * n_cores * 128 >= n_nodes
    g["ROWS_PC"] = w_per_core * 128
    g["REP_ROWS"] = n_cores * g["ROWS_PC"]
    g["N_SRC_WIN"] = 4
    assert g["REP_ROWS"] % 4 == 0
    g["SRC_WIN"] = g["REP_ROWS"] // 4
    assert g["SRC_WIN"] <= 32768
    g["W_PER_SUPER"] = w_per_super
    assert w_per_core % (2 * w_per_super) == 0
    g["SUPERS"] = w_per_core // w_per_super
    g["GROUPS"] = g["SUPERS"] // 2  # 2 supers share one gather instruction
    g["T_PER_BUCKET"] = t_per_bucket
    g["SLOTS_PER_WS"] = t_per_bucket * 128
    # per (group, src-window) gather block: 2 supers * 4 windows * SLOTS_PER_WS
    g["GRP_SLOTS"] = 2 * w_per_super * g["SLOTS_PER_WS"]
    g["GRP_TILES"] = g["GRP_SLOTS"] // 128
    g["SLOTS_TOTAL"] = w_per_core * 4 * g["SLOTS_PER_WS"]
    g["TILES_TOTAL"] = g["SLOTS_TOTAL"] // 128


_config(100000, 10, 8, 104, 4, 3)


# ----------------------------------------------------------------------------
# Host preprocessing
# ----------------------------------------------------------------------------
def _balance_assign(deg_s_fn, tot):
    """Assign nodes to global windows (N_CORES*W_PER_CORE, cap 128 each) so
    that every (window, src-window) edge count stays <= SLOTS_PER_WS."""
    import heapq

    n = tot.shape[0]
    n_windows = N_CORES * W_PER_CORE
    order = np.argsort(-tot, kind="stable")
    heap = [(0, w) for w in range(n_windows)]
    heapq.heapify(heap)
    win_of = np.empty(n, dtype=np.int32)
    win_fill = np.zeros(n_windows, dtype=np.int32)
    for v in order:
        while True:
            load, w = heapq.heappop(heap)
            if win_fill[w] < WIN:
                break
        win_of[v] = w
        win_fill[w] += 1
        if win_fill[w] < WIN:
            heapq.heappush(heap, (load + int(tot[v]), w))

    rng = np.random.default_rng(12345)
    cap = SLOTS_PER_WS
    for round_i in range(12):
        pos_of = np.zeros(n, dtype=np.int32)
        ordv = np.lexsort((np.arange(n), win_of))
        posctr = np.zeros(n_windows, dtype=np.int32)
        for v in ordv:
            pos_of[v] = posctr[win_of[v]]
            posctr[win_of[v]] += 1
        core_of = (win_of // W_PER_CORE).astype(np.int32)
        wloc_of = (win_of % W_PER_CORE).astype(np.int32)
        deg_s = deg_s_fn(core_of, wloc_of, pos_of)  # [n, 4]
        loads = np.zeros((n_windows, N_SRC_WIN), dtype=np.int64)
        np.add.at(loads, win_of, deg_s)
        over = np.flatnonzero((loads > cap).any(axis=1))
        if len(over) == 0:
            return core_of, wloc_of, pos_of
        for w in over:
            s_bad = int(np.argmax(loads[w]))
            excess = int(loads[w, s_bad] - cap)
            members = np.flatnonzero(win_of == w)
            mdeg = deg_s[members, s_bad]
            for v in members[np.argsort(-mdeg)]:
                if excess <= 0:
                    break
                cands = rng.integers(0, n_windows, 64)
                best, bestval = -1, None
                for cw in cands:
                    if cw == w or posctr[cw] >= WIN:
                        continue
                    val = int((loads[cw] + deg_s[v]).max())
                    if val <= cap - 8 and (bestval is None or val < bestval):
                        best, bestval = int(cw), val
                if best < 0:
                    continue
                loads[w] -= deg_s[v]
                loads[best] += deg_s[v]
                win_of[v] = best
                posctr[w] -= 1
                posctr[best] += 1
                excess -= int(deg_s[v, s_bad])
    raise RuntimeError("balance repair failed to converge")


def _preprocess(feats, s, src, dst):
    src = np.asarray(src, dtype=np.int64)
    dst = np.asarray(dst, dtype=np.int64)
    n = N_NODES
    deg = np.bincount(dst, minlength=n).astype(np.float64)
    n2 = (1.0 / deg).astype(np.float32)
    norm = (deg ** -0.5).astype(np.float32)
    sqrtdeg = np.sqrt(deg).astype(np.float32)

    # ---- peel one self-loop per node (handled via identity matmul) ----
    loop_mask = src == dst
    loop_idx = np.flatnonzero(loop_mask)
    uniq_nodes, first_pos = np.unique(dst[loop_idx], return_index=True)
    if len(uniq_nodes) != n:
        raise RuntimeError("not every node has a self-loop; identity fold invalid")
    drop = np.zeros(len(src), dtype=bool)
    drop[loop_idx[first_pos]] = True
    src = src[~drop]
    dst = dst[~drop]

    # ---- node assignment (core, window, pos) ----
    deg_r = np.bincount(dst, minlength=n).astype(np.int64)

    def deg_s_fn(core_of, wloc_of, pos_of):
        rep_row = (
            core_of.astype(np.int64) * ROWS_PC
            + pos_of.astype(np.int64) * W_PER_CORE
            + wloc_of.astype(np.int64)
        )
        es = rep_row[src] // SRC_WIN
        out = np.zeros((n, N_SRC_WIN), dtype=np.int64)
        np.add.at(out, (dst, es), 1)
        return out

    core_of, wloc_of, pos_of = _balance_assign(deg_s_fn, deg_r)
    rep_row = core_of.astype(np.int64) * ROWS_PC + pos_of.astype(np.int64) * W_PER_CORE + wloc_of.astype(np.int64)

    # ---- per-core edge bucketing (group = 2 supers merged per gather) ----
    e_core = core_of[dst]
    e_w = wloc_of[dst]            # window of dst within core (0..W_PER_CORE-1)
    e_key = pos_of[dst]           # indicator column = position of dst in window
    e_srow = rep_row[src]         # replica row of src
    e_s = e_srow // SRC_WIN       # src window id (0..3)
    e_gidx = (e_srow - e_s * SRC_WIN).astype(np.int64)

    gidx_all = np.zeros((N_CORES, SLOTS_TOTAL), dtype=np.int16)
    keys_all = np.full((N_CORES, SLOTS_TOTAL), -1.0, dtype=BF16)

    for c in range(N_CORES):
        m = e_core == c
        cw = e_w[m]
        cs = e_s[m]
        ckey = e_key[m]
        cg = e_gidx[m]
        ws = cw * N_SRC_WIN + cs
        order = np.argsort(ws, kind="stable")
        cw, cs, ckey, cg, ws = cw[order], cs[order], ckey[order], cg[order], ws[order]
        counts = np.bincount(ws, minlength=W_PER_CORE * N_SRC_WIN)
        if counts.max() > SLOTS_PER_WS:
            raise RuntimeError(f"bucket overflow: {counts.max()} > {SLOTS_PER_WS}")
        # slot index of the start of each (w, s) run:
        # group g = w//8, sl = (w%8)//4, wi = w%4
        # slot = ((g*4 + s) * GRP_SLOTS_flat...) layout:
        #   global slot = (g*N_SRC_WIN + s)*GRP_SLOTS + sl*(4*SLOTS_PER_WS)
        #                 + wi*SLOTS_PER_WS + runpos
        w_arr = np.arange(W_PER_CORE * N_SRC_WIN) // N_SRC_WIN
        s_arr = np.arange(W_PER_CORE * N_SRC_WIN) % N_SRC_WIN
        g_arr = w_arr // (2 * W_PER_SUPER)
        sl_arr = (w_arr % (2 * W_PER_SUPER)) // W_PER_SUPER
        wi_arr = w_arr % W_PER_SUPER
        starts = (
            (g_arr * N_SRC_WIN + s_arr) * GRP_SLOTS
            + sl_arr * (W_PER_SUPER * SLOTS_PER_WS)
            + wi_arr * SLOTS_PER_WS
        )
        runpos = np.arange(len(ws)) - np.repeat(
            np.concatenate([[0], np.cumsum(counts)[:-1]]), counts
        )
        slots = starts[ws] + runpos
        gidx_all[c, slots] = cg.astype(np.int16)
        keys_all[c, slots] = ckey.astype(BF16)

    # ---- initial replica g0 = norm * feats in bf16, 256B rows ----
    g0 = (feats.astype(np.float32) * norm[:, None]).astype(BF16)
    g0_rep = np.zeros((REP_ROWS, 2 * D), dtype=BF16)
    g0_rep[rep_row, 0:D] = g0

    def pw_table(vec):  # vec [N] -> [N_CORES, 128, W_PER_CORE]
        out = np.zeros((N_CORES, WIN, W_PER_CORE), dtype=np.float32)
        out[core_of, pos_of, wloc_of] = vec
        return out

    n2_pw = pw_table(n2)
    n2_pw[n2_pw == 0] = 1.0
    sqd_pw = pw_table(sqrtdeg)

    # gather index layout: per (group, s) block of GRP_SLOTS idxs,
    # idx j -> (j%16, j//16), replicated to all 8 gpsimd 16-partition groups
    gidx_wrapped = np.ascontiguousarray(
        np.tile(
            gidx_all.reshape(N_CORES, SLOTS_TOTAL // 16, 16).transpose(0, 2, 1),
            (1, 8, 1),
        )
    )
    # keys layout [128, TILES_TOTAL]: slot j -> (j%128, j//128)
    keys_tiles = np.ascontiguousarray(
        keys_all.reshape(N_CORES, TILES_TOTAL, 128).transpose(0, 2, 1)
    )

    iota = np.broadcast_to(np.arange(128, dtype=np.float32), (128, 128)).astype(BF16)
    iota = np.ascontiguousarray(iota)
    s_bcast = np.broadcast_to(np.asarray(s, dtype=np.float32).reshape(1, D), (128, D))
    s_bcast = np.ascontiguousarray(s_bcast)

    in_maps = []
    for c in range(N_CORES):
        in_maps.append(
            {
                "g0_own": np.ascontiguousarray(
                    g0_rep[c * ROWS_PC : (c + 1) * ROWS_PC]
                ),
                "gidx": gidx_wrapped[c],
                "keys": keys_tiles[c],
                "n2_pw": np.ascontiguousarray(n2_pw[c]),
                "sqd_pw": np.ascontiguousarray(sqd_pw[c]),
                "s_bcast": s_bcast,
                "iota": iota,
            }
        )
    meta = {
        "core_of": core_of,
        "wloc_of": wloc_of,
        "pos_of": pos_of,
    }
    return in_maps, meta


# ----------------------------------------------------------------------------
# Bass kernel builder (identical program for all cores)
# ----------------------------------------------------------------------------
def _build():
    import concourse.bacc as bacc
    import concourse.mybir as mybir
    from concourse.tile import TileContext

    fp32 = mybir.dt.float32
    bf16 = mybir.dt.bfloat16
    i16 = mybir.dt.int16

    nc = bacc.Bacc(None, target_bir_lowering=False, num_devices=N_CORES, num_swdge_queues=4)

    # I/O
    g0_own = nc.dram_tensor("g0_own", [ROWS_PC, 2 * D], bf16, kind="ExternalInput")
    gidx_in = nc.dram_tensor("gidx", [128, SLOTS_TOTAL // 16], i16, kind="ExternalInput")
    keys_in = nc.dram_tensor("keys", [128, TILES_TOTAL], bf16, kind="ExternalInput")
    n2_in = nc.dram_tensor("n2_pw", [128, W_PER_CORE], fp32, kind="ExternalInput")
    sqd_in = nc.dram_tensor("sqd_pw", [128, W_PER_CORE], fp32, kind="ExternalInput")
    s_in = nc.dram_tensor("s_bcast", [128, D], fp32, kind="ExternalInput")
    iota_in = nc.dram_tensor("iota", [128, 128], bf16, kind="ExternalInput")
    out_pm = nc.dram_tensor("out_pm", [ROWS_PC, D], fp32, kind="ExternalOutput")

    cc_in = [
        nc.dram_tensor(f"cc_in_{k}", [ROWS_PC, 2 * D], bf16) for k in range(K_HOPS + 1)
    ]
    cc_out = [
        nc.dram_tensor(f"cc_out_{k}", [REP_ROWS, 2 * D], bf16, addr_space="Shared")
        for k in range(K_HOPS)
    ]
    groups = [list(range(N_CORES))]

    with TileContext(nc) as tc:
        with tc.tile_pool(name="const", bufs=1) as const_pool:
            # ---- load static tables ----
            gidx_sb = const_pool.tile([128, SLOTS_TOTAL // 16], i16, tag="gidx")
            nc.sync.dma_start(out=gidx_sb[:, :], in_=gidx_in[:, :])
            keys_sb = const_pool.tile([128, TILES_TOTAL], bf16, tag="keys")
            nc.sync.dma_start(out=keys_sb[:, :], in_=keys_in[:, :])
            n2_sb = const_pool.tile([128, W_PER_CORE], fp32, tag="n2")
            nc.sync.dma_start(out=n2_sb[:, :], in_=n2_in[:, :])
            sqd_sb = const_pool.tile([128, W_PER_CORE], fp32, tag="sqd")
            nc.sync.dma_start(out=sqd_sb[:, :], in_=sqd_in[:, :])
            s_sb = const_pool.tile([128, D], fp32, tag="svec")
            nc.sync.dma_start(out=s_sb[:, :], in_=s_in[:, :])
            iota_sb = const_pool.tile([128, 128], bf16, tag="iota")
            nc.sync.dma_start(out=iota_sb[:, :], in_=iota_in[:, :])
            ident_sb = const_pool.tile([128, 128], bf16, tag="ident")
            pidx_sb = const_pool.tile([128, 1], fp32, tag="pidx")
            nc.gpsimd.iota(
                pidx_sb[:, :],
                [[1, 1]],
                base=0,
                channel_multiplier=1,
                allow_small_or_imprecise_dtypes=True,
            )
            nc.vector.tensor_scalar(
                ident_sb[:, :],
                iota_sb[:, :],
                pidx_sb[:, :],
                None,
                mybir.AluOpType.is_equal,
            )

            # gating state (lives across the whole hop loop)
            zt_sb = const_pool.tile([128, W_PER_CORE], fp32, tag="zt")
            sig_sb = const_pool.tile([128, W_PER_CORE], fp32, tag="sig")
            acc_sb = const_pool.tile([128, W_PER_CORE, D], fp32, tag="acc")
            uf_sb = const_pool.tile([128, W_PER_CORE, D], fp32, tag="uf")
            nc.vector.memset(acc_sb[:, :, :], 0.0)

            def gate_accum(staged):
                # z[p,w] = sum_f staged[p,w,f]*s[f]; sig = sigmoid(z*sqd);
                # acc += sig * staged
                nc.vector.tensor_tensor(
                    uf_sb[:, :, :],
                    staged[:, :, :],
                    s_sb[:, :]
                    .rearrange("p (one f) -> p one f", one=1)
                    .broadcast_to((128, W_PER_CORE, D)),
                    mybir.AluOpType.mult,
                )
                nc.vector.tensor_reduce(
                    zt_sb[:, :],
                    uf_sb[:, :, :],
                    mybir.AxisListType.X,
                    mybir.AluOpType.add,
                )
                nc.vector.tensor_tensor(
                    zt_sb[:, :], zt_sb[:, :], sqd_sb[:, :], mybir.AluOpType.mult
                )
                nc.scalar.activation(
                    sig_sb[:, :],
                    zt_sb[:, :],
                    mybir.ActivationFunctionType.Sigmoid,
                )
                for w in range(W_PER_CORE):
                    nc.vector.scalar_tensor_tensor(
                        acc_sb[:, w, :],
                        staged[:, w, :],
                        sig_sb[:, w : w + 1],
                        acc_sb[:, w, :],
                        mybir.AluOpType.mult,
                        mybir.AluOpType.add,
                    )

            with (
                tc.tile_pool(name="chunks", bufs=8) as chunk_pool,
                tc.tile_pool(name="inds", bufs=8) as ind_pool,
                tc.tile_pool(name="stage", bufs=2) as stage_pool,
                tc.tile_pool(name="psum", bufs=8, space="PSUM") as psum_pool,
            ):

                # ---- bootstrap: cc_in[0] = g0_own; AllGather -> cc_out[0] ----
                nc.sync.dma_start(out=cc_in[0][:, :], in_=g0_own[:, :])
                nc.gpsimd.collective_compute(
                    "AllGather",
                    mybir.AluOpType.bypass,
                    replica_groups=groups,
                    ins=[cc_in[0][:, :]],
                    outs=[cc_out[0][:, :]],
                )
                staged_prev = stage_pool.tile([128, W_PER_CORE, D], bf16, tag="staged")
                nc.sync.dma_start(
                    out=staged_prev[:, :, :],
                    in_=g0_own[:, 0:D].rearrange("(p w) f -> p w f", p=128),
                )
                gate_accum(staged_prev)

                # ---- hop loop ----
                def drain_super(sup, banks, staged):
                    for wi in range(W_PER_SUPER):
                        w = sup * W_PER_SUPER + wi
                        nc.scalar.activation(
                            staged[:, w, :],
                            banks[wi][:, :],
                            mybir.ActivationFunctionType.Copy,
                            scale=n2_sb[:, w : w + 1],
                        )

                for hop in range(K_HOPS):
                    src_rep = cc_out[hop]
                    staged = stage_pool.tile([128, W_PER_CORE, D], bf16, tag="staged")
                    wt = W_PER_SUPER * T_PER_BUCKET  # 12 tiles per (super, s)
                    pending = None
                    for grp in range(GROUPS):
                        # one gather per src window covering both supers
                        chunks = []
                        for s in range(N_SRC_WIN):
                            ch = chunk_pool.tile(
                                [128, GRP_TILES, 2 * D], bf16, tag="chunk"
                            )
                            col0 = (grp * N_SRC_WIN + s) * (GRP_SLOTS // 16)
                            row0 = s * SRC_WIN
                            nc.gpsimd.dma_gather(
                                ch[:, :, :],
                                src_rep[row0 : row0 + SRC_WIN, :],
                                gidx_sb[:, col0 : col0 + GRP_SLOTS // 16],
                                GRP_SLOTS,
                                GRP_SLOTS,
                                2 * D,
                                single_packet=False,
                                queue_num=s,
                            )
                            chunks.append(ch)
                        for sl in range(2):
                            sup = grp * 2 + sl
                            banks = [
                                psum_pool.tile([128, D], fp32, tag="bank", name="bank")
                                for _ in range(W_PER_SUPER)
                            ]
                            for s in range(N_SRC_WIN):
                                # indicator for (grp, s, sl): 12 tiles
                                tile0 = (grp * N_SRC_WIN + s) * GRP_TILES + sl * wt
                                indb = ind_pool.tile([128, wt, 128], bf16, tag="ind")
                                nc.vector.tensor_tensor(
                                    indb[:, :, :],
                                    iota_sb[:, :]
                                    .rearrange("p (one f) -> p one f", one=1)
                                    .broadcast_to((128, wt, 128)),
                                    keys_sb[:, tile0 : tile0 + wt].broadcast_to(
                                        (128, wt, 128)
                                    ),
                                    mybir.AluOpType.is_equal,
                                )
                                for wi in range(W_PER_SUPER):
                                    w = sup * W_PER_SUPER + wi
                                    bank = banks[wi]
                                    for t in range(T_PER_BUCKET):
                                        j = sl * wt + wi * T_PER_BUCKET + t
                                        nc.tensor.matmul(
                                            bank[:, :],
                                            indb[:, wi * T_PER_BUCKET + t, :],
                                            chunks[s][:, j, 0:D],
                                            start=(s == 0 and t == 0),
                                            stop=False,
                                        )
                                    if s == N_SRC_WIN - 1:
                                        nc.tensor.matmul(
                                            bank[:, :],
                                            ident_sb[:, :],
                                            staged_prev[:, w, :],
                                            start=False,
                                            stop=True,
                                        )
                            if pending is not None:
                                drain_super(pending[0], pending[1], staged)
                            pending = (sup, banks)
                    drain_super(pending[0], pending[1], staged)
                    nc.sync.dma_start(
                        out=cc_in[hop + 1][:, 0:D].rearrange(
                            "(p w) f -> p w f", p=128
                        ),
                        in_=staged[:, :, :],
                    )
                    if hop < K_HOPS - 1:
                        nc.gpsimd.collective_compute(
                            "AllGather",
                            mybir.AluOpType.bypass,
                            replica_groups=groups,
                            ins=[cc_in[hop + 1][:, :]],
                            outs=[cc_out[hop + 1][:, :]],
                        )
                    gate_accum(staged)
                    staged_prev = staged

            # ---- final: out = acc * sqrtdeg ----
            for w in range(W_PER_CORE):
                nc.vector.tensor_scalar_mul(
                    uf_sb[:, w, :], acc_sb[:, w, :], sqd_sb[:, w : w + 1]
                )
            nc.sync.dma_start(
                out=out_pm[:, :].rearrange("(p w) f -> p (w f)", p=128),
                in_=uf_sb[:, :, :],
            )

    nc.finalize()
    return nc


# ----------------------------------------------------------------------------
# Entry point
# ----------------------------------------------------------------------------
_CACHED = {}


def kernel(**inputs):
    feats = np.asarray(inputs["feats"], dtype=np.float32)
    s = np.asarray(inputs["s"], dtype=np.float32)
    src = np.asarray(inputs["src"])
    dst = np.asarray(inputs["dst"])

    in_maps, meta = _preprocess(feats, s, src, dst)

    from concourse.bass_utils import run_bass_kernel_spmd

    nc = _CACHED.get("nc")
    if nc is None:
        nc = _build()
        _CACHED["nc"] = nc

    res = run_bass_kernel_spmd(nc, in_maps, core_ids=list(range(N_CORES)))
    _CACHED["last_result"] = res
    out = np.zeros((N_NODES, D), dtype=np.float32)
    core_of, wloc_of, pos_of = meta["core_of"], meta["wloc_of"], meta["pos_of"]
    rows = pos_of.astype(np.int64) * W_PER_CORE + wloc_of.astype(np.int64)
    for c in range(N_CORES):
        m = core_of == c
        out[m] = res.results[c]["out_pm"][rows[m]]
    return out


if __name__ == "__main__":
    nc = _build()
    print("build ok")
